# revision 5
# baseline (speedup 1.0000x reference)
"""Trainium2 Bass kernel for nn_FBSDE: 8-way data-parallel FBSDE forward pass.

Per core (128 samples): GBM path scan -> lead-lag depth-3 signature features in
a reduced T-tensor basis (2248 cols vs 4368 canonical; the canonical->basis
linear map is folded into Wih on the host) -> two LSTM(512)+MLP heads ->
loss/Y/payoff tail.  Outputs gathered and reduced on host.
"""
import json
import numpy as np

import concourse.bass as bass
import concourse.mybir as mybir
from concourse.tile import TileContext, ScopedClock
from concourse.bass_utils import run_bass_kernel_spmd
from concourse.masks import make_identity

F32, BF16, F32R = mybir.dt.float32, mybir.dt.bfloat16, mybir.dt.float32r
AL = mybir.AluOpType
AF = mybir.ActivationFunctionType

MU, SIGMA = 0.05, 0.2
B, D, NS, LAG, H = 1024, 8, 100, 10, 512
W = NS // LAG + 1           # 11
NC = 8
BC = B // NC                # 128
KPHI = 2304                 # padded phi length (18 k-tiles)
KT = KPHI // 128            # 18
G4 = 4 * H                  # 2048
NWS = W * 128               # 1408

# phi/state layout offsets
O_P1, O_Q2L, O_Q2E, O_USQ, O_T3I, O_T3C = 0, 8, 72, 136, 200, 1736

LET = {0: "B", 1: "A"}
C2 = {("A", "A"): (1., .5), ("A", "B"): (1., 1.), ("B", "A"): (1., 0.), ("B", "B"): (1., .5)}
C3 = {("A", "A", "A"): (1., .5, .5, 1 / 6), ("A", "A", "B"): (1., .5, 1., .5),
      ("A", "B", "A"): (1., 1., 0., 0.), ("A", "B", "B"): (1., 1., .5, .5),
      ("B", "A", "A"): (1., 0., .5, 0.), ("B", "A", "B"): (1., 0., 1., 0.),
      ("B", "B", "A"): (1., .5, 0., 0.), ("B", "B", "B"): (1., .5, .5, 1 / 6)}

ENGINES = {"PE", "DVE", "Activation", "Pool", "SP"}
POOL_WINDOWS = (5, 10)
HALVES = ((0, 1, 2, 3, 4, 5), (6, 7, 8, 9, 10))


def _split_waits(m):
    cnt = 0
    for f in m.get("functions", []):
        for b in f.get("blocks", []):
            new = []
            for inst in b.get("instructions", []):
                si = inst.get("sync_info")
                if si and inst.get("engine") in ENGINES:
                    ws = si.get("on_wait") or []
                    if len(ws) > 1:
                        for wt in ws[:-1]:
                            cnt += 1
                            new.append({"name": f"I-NW{cnt}", "opcode": "NoOp",
                                        "engine": inst["engine"], "ins": [], "outs": [],
                                        "debug": inst.get("debug", 0),
                                        "sync_info": {"on_wait": [wt], "on_update": []}})
                        si["on_wait"] = ws[-1:]
                new.append(inst)
            b["instructions"] = new
    return m


def patch_nc(nc):
    orig = nc.to_json_bytes
    def patched():
        return json.dumps(_split_waits(json.loads(orig()))).encode()
    nc.to_json_bytes = patched
    return nc


class FixTC(TileContext):
    def _drain_and_barrier(self, tick_clock, wait_clock):
        nc = self.nc
        drain_inst = nc.sync.drain()
        wait_clock.add_sem_waits(drain_inst.ins, ScopedClock({None: tick_clock.global_clock}))
        d = drain_inst.ins
        waits = list(d.sync_info.on_wait)
        SI = type(d.sync_info)
        d.sync_info = SI(on_wait=waits[:1], on_update=[])
        for wt in waits[1:]:
            extra = nc.sync.drain()
            extra.ins.sync_info = SI(on_wait=[wt], on_update=[])
        nc.all_engine_barrier()
        popped = nc._tile_sem_poison_stack.pop()
        assert popped is self._sem_poison
        nc.clear_and_free_semaphores(list(self.sems.allocated().values()))
        nc.all_engine_barrier()


def fold_wih(Wih):
    """Wih (2048, 4369) -> Wf (2048, KPHI) in the device phi layout."""
    Gn = Wih.shape[0]
    Wf = np.zeros((Gn, KPHI), np.float64)
    Ws1 = Wih[:, 1:17].astype(np.float64)
    Ws2 = Wih[:, 17:273].astype(np.float64).reshape(Gn, 16, 16)
    Ws3 = Wih[:, 273:4369].astype(np.float64).reshape(Gn, 16, 16, 16)
    for h in (0, 1):
        Wf[:, O_P1:O_P1 + 8] += Ws1[:, h * 8:(h + 1) * 8]
    for h1 in (0, 1):
        for h2 in (0, 1):
            cl, ce = C2[(LET[h1], LET[h2])]
            blk = Ws2[:, h1 * 8:(h1 + 1) * 8, h2 * 8:(h2 + 1) * 8].reshape(Gn, 64)
            Wf[:, O_Q2L:O_Q2L + 64] += cl * blk
            Wf[:, O_Q2E:O_Q2E + 64] += ce * blk
    for h1 in (0, 1):
        for h2 in (0, 1):
            for h3 in (0, 1):
                al, be, ga, ep = C3[(LET[h1], LET[h2], LET[h3])]
                blk = Ws3[:, h1 * 8:(h1 + 1) * 8, h2 * 8:(h2 + 1) * 8, h3 * 8:(h3 + 1) * 8]
                cab = np.transpose(blk, (0, 3, 1, 2)).reshape(Gn, 8, 64)
                abc = blk.reshape(Gn, 8, 64)
                for c in range(8):
                    base = O_T3I + c * 192
                    Wf[:, base:base + 64] += al * cab[:, c]
                    Wf[:, base + 64:base + 128] += be * cab[:, c]
                    Wf[:, base + 128:base + 192] += ep * cab[:, c]
                for a in range(8):
                    Wf[:, O_T3C + a * 64:O_T3C + (a + 1) * 64] += ga * abc[:, a]
    return Wf.astype(np.float32)


def scan_window(nc, eng, state, delta_ap, x0_ap, win, scratch192):
    e = nc.vector if eng == "dve" else nc.gpsimd
    nc.gpsimd.memset(state[:, :], 0.0)
    steps = 1 if win == 0 else LAG
    for j in range(steps):
        u = x0_ap if win == 0 else delta_ap[:, ((win - 1) * LAG + j) * 8:((win - 1) * LAG + j) * 8 + 8]
        ua = u[:, :, None].to_broadcast([BC, 8, 8])
        ub = u[:, None, :].to_broadcast([BC, 8, 8])
        usq_v = state[:, O_USQ:O_USQ + 64].rearrange("p (a b) -> p a b", a=8, b=8)
        e.tensor_tensor(usq_v, ua, ub, AL.mult)
        src192 = state[:, O_Q2L:O_Q2L + 192]   # [Q2l | Q2e | usq]
        usq = state[:, O_USQ:O_USQ + 64]
        if eng == "dve":
            for c in range(8):
                blk = state[:, O_T3I + c * 192:O_T3I + (c + 1) * 192]
                nc.vector.scalar_tensor_tensor(blk, src192, u[:, c:c + 1], blk, AL.mult, AL.add)
            for a in range(8):
                blk = state[:, O_T3C + a * 64:O_T3C + (a + 1) * 64]
                nc.vector.scalar_tensor_tensor(blk, usq, state[:, a:a + 1], blk, AL.mult, AL.add)
        else:
            for c in range(8):
                blk = state[:, O_T3I + c * 192:O_T3I + (c + 1) * 192]
                ucb = u[:, c:c + 1].to_broadcast([BC, 192])
                nc.gpsimd.tensor_tensor(scratch192[:, :], src192, ucb, AL.mult)
                nc.gpsimd.tensor_tensor(blk, blk, scratch192[:, :], AL.add)
            for a in range(8):
                blk = state[:, O_T3C + a * 64:O_T3C + (a + 1) * 64]
                pab = state[:, a:a + 1].to_broadcast([BC, 64])
                nc.gpsimd.tensor_tensor(scratch192[:, 0:64], usq, pab, AL.mult)
                nc.gpsimd.tensor_tensor(blk, blk, scratch192[:, 0:64], AL.add)
        p1a = state[:, 0:8][:, :, None].to_broadcast([BC, 8, 8])
        q2lv = state[:, O_Q2L:O_Q2L + 64].rearrange("p (a b) -> p a b", a=8, b=8)
        tmpv = scratch192[:, 0:64].rearrange("p (a b) -> p a b", a=8, b=8)
        e.tensor_tensor(tmpv, p1a, ub, AL.mult)
        e.tensor_tensor(q2lv, q2lv, tmpv, AL.add)
        e.tensor_tensor(state[:, O_Q2E:O_Q2E + 64], state[:, O_Q2E:O_Q2E + 64], usq, AL.add)
        e.tensor_tensor(state[:, 0:8], state[:, 0:8], u, AL.add)


def build_program(disc):
    nc = bass.Bass()
    di = lambda n, s, dt=F32: nc.dram_tensor(n, s, dt, kind="ExternalInput")
    do = lambda n, s, dt=F32: nc.dram_tensor(n, s, dt, kind="ExternalOutput")
    dint = lambda n, s, dt=F32: nc.dram_tensor(n, s, dt, kind="Internal")

    noise_d = di("noise", (BC, NS * D))
    x0_d = di("x0", (BC, D))
    crows_d = di("crows", (BC, 2 * NS * D))
    bias_d = di("biasg", (128, 2 * 16 * W))
    wf_d = di("wf", (2, 16, 128, KT * 128), F32R)
    whh_d = di("whh", (2, 4, 128, G4), BF16)
    w0_d = di("w0", (2, 4, 128, H), F32R)
    w1_d = di("w1", (2, 4, 128, H), F32R)
    w2f_d = di("w2f", (4, 128, 1), F32R)
    w2g_d = di("w2g", (4, 128, 8), F32R)
    b01_d = di("b01", (128, 16))          # col = layer*8 + net*4 + oc
    b2f_d = di("b2f", (1, 1))
    b2g_d = di("b2g", (8, 1))

    xw_d = dint("xw_s", (2, 16, 128, NWS))
    hT_d = dint("hT_s", (2, 4, 128, NWS), F32R)

    yt_d = do("yt", (1, NWS))
    pay_d = do("pay", (BC, 1))
    wsq_d = do("wsq", (1, W))

    with FixTC(nc) as tc:
        with tc.tile_pool(name="long", bufs=1) as lp:
            delta = lp.tile([BC, NS * D], F32)
            x0_sb = lp.tile([BC, D], F32)
            sinc = lp.tile([BC, (W - 1) * D], F32)
            pay = lp.tile([BC, 1], F32)
            ident = lp.tile([128, 128], F32)
            biasg = lp.tile([128, 2 * 16 * W], F32)
            yt = lp.tile([1, NWS], F32)
            zt = lp.tile([8, NWS], F32)
            nc.sync.dma_start(x0_sb[:], x0_d[:])
            nc.sync.dma_start(biasg[:], bias_d[:])
            make_identity(nc, ident[:])

            # ---------- phase A
            with tc.tile_pool(name="pha", bufs=1) as pa:
                noise = pa.tile([BC, NS * D], F32)
                crows = pa.tile([BC, 2 * NS * D], F32)
                nc.sync.dma_start(noise[:], noise_d[:])
                nc.sync.dma_start(crows[:], crows_d[:])
                dw = pa.tile([BC, NS * D], F32)
                nc.vector.tensor_tensor(dw[:], noise[:], crows[:, :NS * D], AL.mult)
                fct = pa.tile([BC, NS * D], F32)
                nc.vector.scalar_tensor_tensor(fct[:], dw[:], float(SIGMA),
                                               crows[:, NS * D:], AL.mult, AL.add)
                xp = pa.tile([BC, (NS + 1) * D], F32)
                nc.vector.tensor_copy(xp[:, 0:D], x0_sb[:])
                for t in range(NS):
                    nc.vector.tensor_tensor(xp[:, (t + 1) * D:(t + 2) * D],
                                            xp[:, t * D:(t + 1) * D],
                                            fct[:, t * D:(t + 1) * D], AL.mult)
                nc.vector.tensor_tensor(delta[:], xp[:, D:], xp[:, :NS * D], AL.subtract)
                bsk = pa.tile([BC, NS + 1], F32)
                nc.vector.tensor_reduce(bsk[:], xp[:].rearrange("p (t d) -> p t d", d=D),
                                        mybir.AxisListType.X, AL.add)
                bmax = pa.tile([BC, 1], F32)
                nc.vector.tensor_reduce(bmax[:], bsk[:], mybir.AxisListType.X, AL.max)
                nc.vector.tensor_tensor(pay[:], bmax[:], bsk[:, NS:NS + 1], AL.subtract)
                nc.vector.tensor_scalar_mul(pay[:], pay[:], 1.0 / D)
                nc.sync.dma_start(pay_d[:], pay[:])
                nc.vector.tensor_reduce(sinc[:].rearrange("p (w d) -> p w d", d=D),
                                        dw[:].rearrange("p (w t d) -> p w d t", w=W - 1, t=LAG, d=D),
                                        mybir.AxisListType.X, AL.add)

            # ---------- phases B+C: signatures + xW
            with tc.tile_pool(name="phb", bufs=1) as pb, \
                 tc.tile_pool(name="wtile", bufs=2) as wtp, \
                 tc.tile_pool(name="xstage", bufs=2) as xsp, \
                 tc.tile_pool(name="tps", bufs=2, space="PSUM") as tpp, \
                 tc.tile_pool(name="xps", bufs=2, space="PSUM") as xpp:
                state_dve = pb.tile([BC, KPHI], F32, tag="st_d")
                state_pool = pb.tile([BC, KPHI], F32, tag="st_p")
                scr_d = pb.tile([BC, 192], F32)
                scr_p = pb.tile([BC, 192], F32)
                for half, wins in enumerate(HALVES):
                    nwin = len(wins)
                    nw = nwin * 128
                    phiT = {k: pb.tile([128, 6 * 128], F32R, tag=f"phiT{k}", name=f"phiT{k}") for k in range(KT)}
                    for wloc, win in enumerate(wins):
                        if win in POOL_WINDOWS:
                            st, scr, eng = state_pool, scr_p, "pool"
                        else:
                            st, scr, eng = state_dve, scr_d, "dve"
                        scan_window(nc, eng, st, delta[:], x0_sb[:], win, scr)
                        for k in range(KT):
                            ps = tpp.tile([128, 128], F32, tag="tp")
                            nc.tensor.transpose(ps[:], st[:, k * 128:(k + 1) * 128], ident[:])
                            nc.scalar.copy(phiT[k][:, wloc * 128:(wloc + 1) * 128], ps[:])
                    for net in range(2):
                        for gc in range(16):
                            wt = wtp.tile([128, KT * 128], F32R, tag="wt")
                            nc.sync.dma_start(wt[:], wf_d[net, gc])
                            psxa = xpp.tile([128, 384], F32, tag="psxa")
                            psxb = xpp.tile([128, 384], F32, tag="psxb")
                            for k in range(KT):
                                nc.tensor.matmul(psxa[:, :], wt[:, k * 128:(k + 1) * 128],
                                                 phiT[k][:, 0:384],
                                                 start=(k == 0), stop=(k == KT - 1))
                            for k in range(KT):
                                nc.tensor.matmul(psxb[:, :nw - 384], wt[:, k * 128:(k + 1) * 128],
                                                 phiT[k][:, 384:nw],
                                                 start=(k == 0), stop=(k == KT - 1))
                            xs = xsp.tile([128, 6 * 128], F32, tag="xs")
                            nc.scalar.copy(xs[:, 0:384], psxa[:, :])
                            nc.scalar.copy(xs[:, 384:nw], psxb[:, :nw - 384])
                            nc.sync.dma_start(
                                xw_d[net, gc][:, wins[0] * 128:(wins[-1] + 1) * 128],
                                xs[:, :nw])

            # ---------- phase D: recurrence
            whh_sb = lp.tile([128, 2 * 4 * G4], BF16)
            for net in range(2):
                for j in range(4):
                    nc.sync.dma_start(whh_sb[:, (net * 4 + j) * G4:(net * 4 + j + 1) * G4],
                                      whh_d[net, j])
            cT = lp.tile([128, 2 * 4 * 128], F32)
            hTb = lp.tile([128, 2 * 4 * 128], BF16)
            nc.gpsimd.memset(cT[:], 0.0)
            with tc.tile_pool(name="phd", bufs=2) as pd, \
                 tc.tile_pool(name="hst", bufs=4) as hst, \
                 tc.tile_pool(name="rps", bufs=2, space="PSUM") as rpp:
                for w in range(W):
                    for net in range(2):
                        xwt = pd.tile([128, 16 * 128], F32, tag="xw")
                        for gc in range(16):
                            nc.sync.dma_start(xwt[:, gc * 128:(gc + 1) * 128],
                                              xw_d[net, gc][:, w * 128:(w + 1) * 128])
                        gats = pd.tile([128, 16 * 128], F32, tag="gats")
                        if w > 0:
                            psg = rpp.tile([128, 16 * 128], F32, tag="psg")
                            for gc in range(16):
                                for j in range(4):
                                    base = (net * 4 + j) * G4
                                    nc.tensor.matmul(
                                        psg[:, gc * 128:(gc + 1) * 128],
                                        whh_sb[:, base + gc * 128:base + (gc + 1) * 128],
                                        hTb[:, (net * 4 + j) * 128:(net * 4 + j + 1) * 128],
                                        start=(j == 0), stop=(j == 3))
                            for gc in range(16):
                                bcol = biasg[:, net * 176 + gc * 11 + w:net * 176 + gc * 11 + w + 1]
                                nc.vector.scalar_tensor_tensor(
                                    gats[:, gc * 128:(gc + 1) * 128],
                                    psg[:, gc * 128:(gc + 1) * 128],
                                    bcol, xwt[:, gc * 128:(gc + 1) * 128], AL.add, AL.add)
                        else:
                            for gc in range(16):
                                bcol = biasg[:, net * 176 + gc * 11 + w:net * 176 + gc * 11 + w + 1]
                                nc.vector.tensor_scalar(gats[:, gc * 128:(gc + 1) * 128],
                                                        xwt[:, gc * 128:(gc + 1) * 128],
                                                        bcol, None, AL.add)
                        acts = pd.tile([128, 16 * 128], F32, tag="acts")
                        nc.scalar.activation(acts[:, 0:1024], gats[:, 0:1024], AF.Sigmoid)
                        nc.scalar.activation(acts[:, 1024:1536], gats[:, 1024:1536], AF.Tanh)
                        nc.scalar.activation(acts[:, 1536:2048], gats[:, 1536:2048], AF.Sigmoid)
                        for j in range(4):
                            co = (net * 4 + j) * 128
                            ig = acts[:, j * 128:(j + 1) * 128]
                            fg = acts[:, 512 + j * 128:512 + (j + 1) * 128]
                            gg = acts[:, 1024 + j * 128:1024 + (j + 1) * 128]
                            og = acts[:, 1536 + j * 128:1536 + (j + 1) * 128]
                            nc.vector.tensor_tensor(cT[:, co:co + 128], fg, cT[:, co:co + 128], AL.mult)
                            tmpc = hst.tile([128, 128], F32, tag="tmpc")
                            nc.vector.tensor_tensor(tmpc[:], ig, gg, AL.mult)
                            nc.vector.tensor_tensor(cT[:, co:co + 128], cT[:, co:co + 128],
                                                    tmpc[:], AL.add)
                            th = hst.tile([128, 128], F32, tag="th")
                            nc.scalar.activation(th[:], cT[:, co:co + 128], AF.Tanh)
                            hT = hst.tile([128, 128], F32R, tag="hT")
                            nc.vector.tensor_tensor(hT[:], og, th[:], AL.mult)
                            nc.vector.tensor_copy(hTb[:, co:co + 128], hT[:].bitcast(F32))
                            nc.sync.dma_start(hT_d[net, j][:, w * 128:(w + 1) * 128], hT[:])

            # ---------- phase E: FFN heads
            nch = ((0, 512), (512, 1024), (1024, NWS))
            b01sb = lp.tile([128, 16], F32)
            nc.sync.dma_start(b01sb[:], b01_d[:])
            b2fsb = lp.tile([1, 1], F32)
            nc.sync.dma_start(b2fsb[:], b2f_d[:])
            b2gsb = lp.tile([8, 1], F32)
            nc.sync.dma_start(b2gsb[:], b2g_d[:])
            with tc.tile_pool(name="phe", bufs=1) as pe, \
                 tc.tile_pool(name="fps", bufs=2, space="PSUM") as fpp:
                for net in range(2):
                    hsb = pe.tile([128, 4 * NWS], F32R, tag="hsb")
                    for j in range(4):
                        nc.sync.dma_start(hsb[:, j * NWS:(j + 1) * NWS], hT_d[net, j])
                    w0sb = pe.tile([128, 4 * H], F32R, tag="w0sb")
                    w1sb = pe.tile([128, 4 * H], F32R, tag="w1sb")
                    for j in range(4):
                        nc.sync.dma_start(w0sb[:, j * H:(j + 1) * H], w0_d[net, j])
                        nc.sync.dma_start(w1sb[:, j * H:(j + 1) * H], w1_d[net, j])
                    o1 = pe.tile([128, 4 * NWS], F32R, tag="o1")
                    for oc in range(4):
                        for (n0, n1) in nch:
                            ps1 = fpp.tile([128, 512], F32, tag="ps1")
                            for j in range(4):
                                nc.tensor.matmul(ps1[:, :n1 - n0],
                                                 w0sb[:, j * H + oc * 128:j * H + (oc + 1) * 128],
                                                 hsb[:, j * NWS + n0:j * NWS + n1],
                                                 start=(j == 0), stop=(j == 3))
                            nc.scalar.activation(o1[:, oc * NWS + n0:oc * NWS + n1],
                                                 ps1[:, :n1 - n0], AF.Relu,
                                                 bias=b01sb[:, net * 4 + oc:net * 4 + oc + 1])
                    o2 = pe.tile([128, 4 * NWS], F32R, tag="o2")
                    for oc in range(4):
                        for (n0, n1) in nch:
                            ps2 = fpp.tile([128, 512], F32, tag="ps2")
                            for j in range(4):
                                nc.tensor.matmul(ps2[:, :n1 - n0],
                                                 w1sb[:, j * H + oc * 128:j * H + (oc + 1) * 128],
                                                 o1[:, j * NWS + n0:j * NWS + n1],
                                                 start=(j == 0), stop=(j == 3))
                            nc.scalar.activation(o2[:, oc * NWS + n0:oc * NWS + n1],
                                                 ps2[:, :n1 - n0], AF.Relu,
                                                 bias=b01sb[:, 8 + net * 4 + oc:8 + net * 4 + oc + 1])
                    if net == 0:
                        w2sb = pe.tile([128, 4], F32R, tag="w2f")
                        for j in range(4):
                            nc.sync.dma_start(w2sb[:, j:j + 1], w2f_d[j])
                        for (n0, n1) in nch:
                            ps3 = fpp.tile([1, 512], F32, tag="ps3f")
                            for j in range(4):
                                nc.tensor.matmul(ps3[:, :n1 - n0], w2sb[:, j:j + 1],
                                                 o2[:, j * NWS + n0:j * NWS + n1],
                                                 start=(j == 0), stop=(j == 3))
                            nc.vector.tensor_scalar(yt[:, n0:n1], ps3[:, :n1 - n0],
                                                    b2fsb[:, 0:1], None, AL.add)
                    else:
                        w2sb = pe.tile([128, 4 * 8], F32R, tag="w2g")
                        for j in range(4):
                            nc.sync.dma_start(w2sb[:, j * 8:(j + 1) * 8], w2g_d[j])
                        for (n0, n1) in nch:
                            ps3 = fpp.tile([8, 512], F32, tag="ps3g")
                            for j in range(4):
                                nc.tensor.matmul(ps3[:, :n1 - n0], w2sb[:, j * 8:(j + 1) * 8],
                                                 o2[:, j * NWS + n0:j * NWS + n1],
                                                 start=(j == 0), stop=(j == 3))
                            nc.vector.tensor_scalar(zt[:, n0:n1], ps3[:, :n1 - n0],
                                                    b2gsb[:, 0:1], None, AL.add)
            nc.sync.dma_start(yt_d[:], yt[:])

            # ---------- phase F: tail
            with tc.tile_pool(name="phf", bufs=1) as pf, \
                 tc.tile_pool(name="tps2", bufs=2, space="PSUM") as tp2:
                sincT = pf.tile([8, NWS], F32)
                nc.gpsimd.memset(sincT[:, (W - 1) * 128:], 0.0)
                for w in range(W - 1):
                    pst = tp2.tile([8, 128], F32, tag="tps")
                    nc.tensor.transpose(pst[:], sinc[:, w * D:(w + 1) * D], ident[:])
                    nc.scalar.copy(sincT[:, w * 128:(w + 1) * 128], pst[:])
                zs = pf.tile([8, NWS], F32R)
                nc.vector.tensor_tensor(zs[:], zt[:], sincT[:], AL.mult)
                ones8f = pf.tile([8, 1], F32)
                nc.gpsimd.memset(ones8f[:], 1.0)
                ones8 = pf.tile([8, 1], F32R)
                nc.vector.tensor_copy(ones8[:], ones8f[:])
                pred = pf.tile([1, NWS], F32)
                for (n0, n1) in ((0, 512), (512, 1024), (1024, NWS)):
                    psz = tp2.tile([1, 512], F32, tag="psz")
                    nc.tensor.matmul(psz[:, :n1 - n0], ones8[:], zs[:, n0:n1],
                                     start=True, stop=True)
                    nc.vector.tensor_tensor(pred[:, n0:n1], yt[:, n0:n1],
                                            psz[:, :n1 - n0], AL.add)
                targ = pf.tile([1, NWS], F32)
                for w in range(W - 1):
                    nc.vector.tensor_scalar_mul(targ[:, w * 128:(w + 1) * 128],
                                                yt[:, (w + 1) * 128:(w + 2) * 128], float(disc[w]))
                pstp = tp2.tile([1, 128], F32, tag="tpp2")
                nc.tensor.transpose(pstp[:], pay[:], ident[:])
                nc.scalar.copy(targ[:, (W - 1) * 128:], pstp[:])
                diff = pf.tile([1, NWS], F32)
                nc.vector.tensor_tensor(diff[:], pred[:], targ[:], AL.subtract)
                nc.vector.tensor_tensor(diff[:], diff[:], diff[:], AL.mult)
                wsq = pf.tile([1, W], F32)
                nc.vector.tensor_reduce(wsq[:], diff[:].rearrange("p (w s) -> p w s", w=W),
                                        mybir.AxisListType.X, AL.add)
                nc.sync.dma_start(wsq_d[:], wsq[:])

    patch_nc(nc)
    return nc


_CACHE = {}


def _host_prep(inputs):
    import ml_dtypes
    ts = np.asarray(inputs["ts"], np.float32)
    h_t = ts[1:] - ts[:-1]
    sqrt_h = np.sqrt(h_t)
    tcg = ts[::LAG]
    disc = np.exp(-MU * (ts[np.arange(W - 1) + LAG] - tcg[:W - 1])).astype(np.float64)

    wf_host = np.zeros((2, 16, 128, KT * 128), np.float32)
    bias_host = np.zeros((128, 2 * 16 * W), np.float32)
    whh_host = np.zeros((2, 4, 128, G4), np.float32)
    w0_host = np.zeros((2, 4, 128, H), np.float32)
    w1_host = np.zeros((2, 4, 128, H), np.float32)
    b01_host = np.zeros((128, 16), np.float32)
    for n, pre in enumerate(("f", "g")):
        Wih = np.asarray(inputs[pre + "_Wih"], np.float32)
        Wf = fold_wih(Wih)
        v = Wf.reshape(16, 128, KT, 128)                      # [gc][c][j][p]
        wf_host[n] = np.ascontiguousarray(v.transpose(0, 3, 2, 1)).reshape(16, 128, KT * 128)
        bias = (np.asarray(inputs[pre + "_bih"], np.float32)
                + np.asarray(inputs[pre + "_bhh"], np.float32))[None, :] \
            + tcg[:, None] * Wih[:, 0][None, :]
        bb = bias.reshape(W, 16, 128)                          # [w][gc][p]
        bias_host[:, n * 176:(n + 1) * 176] = bb.transpose(2, 1, 0).reshape(128, 176)
        whh_host[n] = np.asarray(inputs[pre + "_Whh"], np.float32).T.reshape(4, 128, G4)
        w0_host[n] = np.asarray(inputs[pre + "_W0"], np.float32).T.reshape(4, 128, H)
        w1_host[n] = np.asarray(inputs[pre + "_W1"], np.float32).T.reshape(4, 128, H)
        b01_host[:, n * 4:(n + 1) * 4] = np.asarray(inputs[pre + "_b0"], np.float32).reshape(4, 128).T
        b01_host[:, 8 + n * 4:8 + (n + 1) * 4] = np.asarray(inputs[pre + "_b1"], np.float32).reshape(4, 128).T
    whh_host = whh_host.astype(ml_dtypes.bfloat16)
    w2f_host = np.ascontiguousarray(np.asarray(inputs["f_W2"], np.float32).T.reshape(4, 128, 1))
    w2g_host = np.ascontiguousarray(np.asarray(inputs["g_W2"], np.float32).T).reshape(4, 128, 8)
    b2f_host = np.asarray(inputs["f_b2"], np.float32).reshape(1, 1)
    b2g_host = np.asarray(inputs["g_b2"], np.float32).reshape(8, 1)

    crow = np.zeros((BC, 2 * NS * D), np.float32)
    crow[:, :NS * D] = np.repeat(sqrt_h, D)[None, :]
    crow[:, NS * D:] = np.repeat(1.0 + MU * h_t, D)[None, :]

    shared = dict(crows=crow, biasg=bias_host, wf=wf_host, whh=whh_host,
                  w0=w0_host, w1=w1_host, w2f=w2f_host, w2g=w2g_host,
                  b01=b01_host, b2f=b2f_host, b2g=b2g_host)
    return disc, shared


def kernel(**inputs):
    x0 = np.ascontiguousarray(np.asarray(inputs["x0"], np.float32))
    noise = np.ascontiguousarray(np.asarray(inputs["noise"], np.float32))
    disc, shared = _host_prep(inputs)

    if "prog" not in _CACHE:
        _CACHE["prog"] = build_program(disc)
    nc = _CACHE["prog"]

    in_maps = []
    for c in range(NC):
        sl = slice(c * BC, (c + 1) * BC)
        in_maps.append(dict(shared, noise=noise[sl].reshape(BC, NS * D), x0=x0[sl]))

    res = run_bass_kernel_spmd(nc, in_maps, core_ids=list(range(NC)))

    Y = np.zeros((B, W, 1), np.float32)
    pay = np.zeros((B, 1), np.float32)
    wsq = np.zeros(W, np.float64)
    for c in range(NC):
        r = res.results[c]
        Y[c * BC:(c + 1) * BC] = r["yt"].reshape(W, BC).T[:, :, None]
        pay[c * BC:(c + 1) * BC] = r["pay"]
        wsq += r["wsq"][0].astype(np.float64)
    loss = np.float32((wsq / B).sum())
    return loss, Y, pay


# revision 8
# speedup vs baseline: 1.1931x; 1.1931x over previous
"""Trainium2 Bass kernel for nn_FBSDE: 8-way data-parallel FBSDE forward pass.

Per core (128 samples): GBM path scan -> lead-lag depth-3 signature features in
a reduced T-tensor basis (2248 cols vs 4368 canonical; the canonical->basis
linear map is folded into Wih on the host) -> two LSTM(512)+MLP heads ->
loss/Y/payoff tail.  Outputs gathered and reduced on host.
"""
import json
import numpy as np

import concourse.bass as bass
import concourse.mybir as mybir
from concourse.tile import TileContext, ScopedClock
from concourse.bass_utils import run_bass_kernel_spmd
from concourse.masks import make_identity

F32, BF16, F32R = mybir.dt.float32, mybir.dt.bfloat16, mybir.dt.float32r
AL = mybir.AluOpType
AF = mybir.ActivationFunctionType

MU, SIGMA = 0.05, 0.2
B, D, NS, LAG, H = 1024, 8, 100, 10, 512
W = NS // LAG + 1           # 11
NC = 8
BC = B // NC                # 128
KPHI = 2304                 # padded phi length (18 k-tiles)
KT = KPHI // 128            # 18
G4 = 4 * H                  # 2048
NWS = W * 128               # 1408

# phi/state layout offsets
O_P1, O_Q2L, O_Q2E, O_USQ, O_T3I, O_T3C = 0, 8, 72, 136, 200, 1736
NPHI = 2250  # last two cols: const-1 and t features (bias folded into Wf)

LET = {0: "B", 1: "A"}
C2 = {("A", "A"): (1., .5), ("A", "B"): (1., 1.), ("B", "A"): (1., 0.), ("B", "B"): (1., .5)}
C3 = {("A", "A", "A"): (1., .5, .5, 1 / 6), ("A", "A", "B"): (1., .5, 1., .5),
      ("A", "B", "A"): (1., 1., 0., 0.), ("A", "B", "B"): (1., 1., .5, .5),
      ("B", "A", "A"): (1., 0., .5, 0.), ("B", "A", "B"): (1., 0., 1., 0.),
      ("B", "B", "A"): (1., .5, 0., 0.), ("B", "B", "B"): (1., .5, .5, 1 / 6)}

ENGINES = {"PE", "DVE", "Activation", "Pool", "SP"}
POOL_WINDOWS = (5, 10)
GROUPS = ((0, 1, 2, 3), (4, 5, 6, 7), (8, 9, 10))


def _split_waits(m):
    cnt = 0
    for f in m.get("functions", []):
        for b in f.get("blocks", []):
            new = []
            for inst in b.get("instructions", []):
                si = inst.get("sync_info")
                if si and inst.get("engine") in ENGINES:
                    ws = si.get("on_wait") or []
                    if len(ws) > 1:
                        for wt in ws[:-1]:
                            cnt += 1
                            new.append({"name": f"I-NW{cnt}", "opcode": "NoOp",
                                        "engine": inst["engine"], "ins": [], "outs": [],
                                        "debug": inst.get("debug", 0),
                                        "sync_info": {"on_wait": [wt], "on_update": []}})
                        si["on_wait"] = ws[-1:]
                new.append(inst)
            b["instructions"] = new
    return m


def patch_nc(nc):
    orig = nc.to_json_bytes
    def patched():
        return json.dumps(_split_waits(json.loads(orig()))).encode()
    nc.to_json_bytes = patched
    return nc


class FixTC(TileContext):
    def _drain_and_barrier(self, tick_clock, wait_clock):
        nc = self.nc
        drain_inst = nc.sync.drain()
        wait_clock.add_sem_waits(drain_inst.ins, ScopedClock({None: tick_clock.global_clock}))
        d = drain_inst.ins
        waits = list(d.sync_info.on_wait)
        SI = type(d.sync_info)
        d.sync_info = SI(on_wait=waits[:1], on_update=[])
        for wt in waits[1:]:
            extra = nc.sync.drain()
            extra.ins.sync_info = SI(on_wait=[wt], on_update=[])
        nc.all_engine_barrier()
        popped = nc._tile_sem_poison_stack.pop()
        assert popped is self._sem_poison
        nc.clear_and_free_semaphores(list(self.sems.allocated().values()))
        nc.all_engine_barrier()


def fold_wih(Wih):
    """Wih (2048, 4369) -> Wf (2048, KPHI) in the device phi layout."""
    Gn = Wih.shape[0]
    Wf = np.zeros((Gn, KPHI), np.float64)
    Ws1 = Wih[:, 1:17].astype(np.float64)
    Ws2 = Wih[:, 17:273].astype(np.float64).reshape(Gn, 16, 16)
    Ws3 = Wih[:, 273:4369].astype(np.float64).reshape(Gn, 16, 16, 16)
    for h in (0, 1):
        Wf[:, O_P1:O_P1 + 8] += Ws1[:, h * 8:(h + 1) * 8]
    for h1 in (0, 1):
        for h2 in (0, 1):
            cl, ce = C2[(LET[h1], LET[h2])]
            blk = Ws2[:, h1 * 8:(h1 + 1) * 8, h2 * 8:(h2 + 1) * 8].reshape(Gn, 64)
            Wf[:, O_Q2L:O_Q2L + 64] += cl * blk
            Wf[:, O_Q2E:O_Q2E + 64] += ce * blk
    for h1 in (0, 1):
        for h2 in (0, 1):
            for h3 in (0, 1):
                al, be, ga, ep = C3[(LET[h1], LET[h2], LET[h3])]
                blk = Ws3[:, h1 * 8:(h1 + 1) * 8, h2 * 8:(h2 + 1) * 8, h3 * 8:(h3 + 1) * 8]
                cab = np.transpose(blk, (0, 3, 1, 2)).reshape(Gn, 8, 64)
                abc = blk.reshape(Gn, 8, 64)
                for c in range(8):
                    base = O_T3I + c * 192
                    Wf[:, base:base + 64] += al * cab[:, c]
                    Wf[:, base + 64:base + 128] += be * cab[:, c]
                    Wf[:, base + 128:base + 192] += ep * cab[:, c]
                for a in range(8):
                    Wf[:, O_T3C + a * 64:O_T3C + (a + 1) * 64] += ga * abc[:, a]
    return Wf.astype(np.float32)


def scan_window(nc, eng, state, delta_ap, x0_ap, win, scratch, tcw):
    e = nc.vector if eng == "dve" else nc.gpsimd
    nc.gpsimd.memset(state[:, :], 0.0)
    nc.gpsimd.memset(state[:, NPHI - 2:NPHI - 1], 1.0)      # const-1 feature (bih+bhh)
    nc.gpsimd.memset(state[:, NPHI - 1:NPHI], float(tcw))   # t feature (Wih[:,0])
    steps = 1 if win == 0 else LAG
    for j in range(steps):
        u = x0_ap if win == 0 else delta_ap[:, ((win - 1) * LAG + j) * 8:((win - 1) * LAG + j) * 8 + 8]
        ua = u[:, :, None].to_broadcast([BC, 8, 8])
        ub = u[:, None, :].to_broadcast([BC, 8, 8])
        usq_v = state[:, O_USQ:O_USQ + 64].rearrange("p (a b) -> p a b", a=8, b=8)
        e.tensor_tensor(usq_v, ua, ub, AL.mult)
        src192 = state[:, O_Q2L:O_Q2L + 192]   # [Q2l | Q2e | usq]
        usq = state[:, O_USQ:O_USQ + 64]
        if eng == "dve":
            for c in range(8):
                blk = state[:, O_T3I + c * 192:O_T3I + (c + 1) * 192]
                nc.vector.scalar_tensor_tensor(blk, src192, u[:, c:c + 1], blk, AL.mult, AL.add)
            for a in range(8):
                blk = state[:, O_T3C + a * 64:O_T3C + (a + 1) * 64]
                nc.vector.scalar_tensor_tensor(blk, usq, state[:, a:a + 1], blk, AL.mult, AL.add)
        else:
            src_b = src192[:, None, :].to_broadcast([BC, 8, 192])
            u_b = u[:, :, None].to_broadcast([BC, 8, 192])
            t3i = state[:, O_T3I:O_T3I + 1536].rearrange("p (c f) -> p c f", c=8, f=192)
            scv = scratch[:, 0:1536].rearrange("p (c f) -> p c f", c=8, f=192)
            nc.gpsimd.tensor_tensor(scv, src_b, u_b, AL.mult)
            nc.gpsimd.tensor_tensor(t3i, t3i, scv, AL.add)
            usq_b = usq[:, None, :].to_broadcast([BC, 8, 64])
            p1_b = state[:, 0:8][:, :, None].to_broadcast([BC, 8, 64])
            t3c = state[:, O_T3C:O_T3C + 512].rearrange("p (a f) -> p a f", a=8, f=64)
            scv2 = scratch[:, 0:512].rearrange("p (a f) -> p a f", a=8, f=64)
            nc.gpsimd.tensor_tensor(scv2, usq_b, p1_b, AL.mult)
            nc.gpsimd.tensor_tensor(t3c, t3c, scv2, AL.add)
        p1a = state[:, 0:8][:, :, None].to_broadcast([BC, 8, 8])
        q2lv = state[:, O_Q2L:O_Q2L + 64].rearrange("p (a b) -> p a b", a=8, b=8)
        tmpv = scratch[:, 0:64].rearrange("p (a b) -> p a b", a=8, b=8)
        e.tensor_tensor(tmpv, p1a, ub, AL.mult)
        e.tensor_tensor(q2lv, q2lv, tmpv, AL.add)
        e.tensor_tensor(state[:, O_Q2E:O_Q2E + 64], state[:, O_Q2E:O_Q2E + 64], usq, AL.add)
        e.tensor_tensor(state[:, 0:8], state[:, 0:8], u, AL.add)


def build_program(disc, tcg):
    nc = bass.Bass()
    di = lambda n, s, dt=F32: nc.dram_tensor(n, s, dt, kind="ExternalInput")
    do = lambda n, s, dt=F32: nc.dram_tensor(n, s, dt, kind="ExternalOutput")
    dint = lambda n, s, dt=F32: nc.dram_tensor(n, s, dt, kind="Internal")

    noise_d = di("noise", (BC, NS * D))
    x0_d = di("x0", (BC, D))
    crows_d = di("crows", (BC, 2 * NS * D))
    wf_d = di("wf", (2, 16, 128, KT * 128), F32R)
    whh_d = di("whh", (2, 4, 128, G4), BF16)
    w0_d = di("w0", (2, 4, 128, H), F32R)
    w1_d = di("w1", (2, 4, 128, H), F32R)
    w2f_d = di("w2f", (4, 128, 1), F32R)
    w2g_d = di("w2g", (4, 128, 8), F32R)
    b01_d = di("b01", (128, 16))
    b2f_d = di("b2f", (1, 1))
    b2g_d = di("b2g", (8, 1))

    xw_d = dint("xw_s", (2, W, 128, G4))
    hT_d = dint("hT_s", (2, 4, 128, NWS), F32R)

    yt_d = do("yt", (1, NWS))
    pay_d = do("pay", (BC, 1))
    wsq_d = do("wsq", (1, W))

    with FixTC(nc) as tc:
        with tc.tile_pool(name="long", bufs=1) as lp:
            delta = lp.tile([BC, NS * D], F32)
            x0_sb = lp.tile([BC, D], F32)
            sinc = lp.tile([BC, (W - 1) * D], F32)
            pay = lp.tile([BC, 1], F32)
            ident = lp.tile([128, 128], F32)
            yt = lp.tile([1, NWS], F32)
            zt = lp.tile([8, NWS], F32)
            nc.sync.dma_start(x0_sb[:], x0_d[:])
            make_identity(nc, ident[:])

            # ---------- phase A
            with tc.tile_pool(name="pha", bufs=1) as pa:
                noise = pa.tile([BC, NS * D], F32)
                crows = pa.tile([BC, 2 * NS * D], F32)
                nc.sync.dma_start(noise[:], noise_d[:])
                nc.sync.dma_start(crows[:], crows_d[:])
                dw = pa.tile([BC, NS * D], F32)
                nc.vector.tensor_tensor(dw[:], noise[:], crows[:, :NS * D], AL.mult)
                fct = pa.tile([BC, NS * D], F32)
                nc.vector.scalar_tensor_tensor(fct[:], dw[:], float(SIGMA),
                                               crows[:, NS * D:], AL.mult, AL.add)
                xp = pa.tile([BC, (NS + 1) * D], F32)
                nc.vector.tensor_copy(xp[:, 0:D], x0_sb[:])
                for t in range(NS):
                    nc.vector.tensor_tensor(xp[:, (t + 1) * D:(t + 2) * D],
                                            xp[:, t * D:(t + 1) * D],
                                            fct[:, t * D:(t + 1) * D], AL.mult)
                nc.vector.tensor_tensor(delta[:], xp[:, D:], xp[:, :NS * D], AL.subtract)
                bsk = pa.tile([BC, NS + 1], F32)
                nc.vector.tensor_reduce(bsk[:], xp[:].rearrange("p (t d) -> p t d", d=D),
                                        mybir.AxisListType.X, AL.add)
                bmax = pa.tile([BC, 1], F32)
                nc.vector.tensor_reduce(bmax[:], bsk[:], mybir.AxisListType.X, AL.max)
                nc.vector.tensor_tensor(pay[:], bmax[:], bsk[:, NS:NS + 1], AL.subtract)
                nc.vector.tensor_scalar_mul(pay[:], pay[:], 1.0 / D)
                nc.sync.dma_start(pay_d[:], pay[:])
                nc.vector.tensor_reduce(sinc[:].rearrange("p (w d) -> p w d", d=D),
                                        dw[:].rearrange("p (w t d) -> p w d t", w=W - 1, t=LAG, d=D),
                                        mybir.AxisListType.X, AL.add)

            # ---------- phases B+C: signatures + xW, pipelined by window group
            with tc.tile_pool(name="phb", bufs=1) as pb, \
                 tc.tile_pool(name="wtile", bufs=3) as wtp, \
                 tc.tile_pool(name="xstage", bufs=3) as xsp, \
                 tc.tile_pool(name="tps", bufs=4, space="PSUM") as tpp, \
                 tc.tile_pool(name="xps", bufs=2, space="PSUM") as xpp, \
                 tc.tile_pool(name="phiT", bufs=2) as ptp:
                state_dve = pb.tile([BC, KPHI], F32, tag="st_d")
                state_pool = pb.tile([BC, KPHI], F32, tag="st_p")
                scr_d = pb.tile([BC, 192], F32)
                scr_p = pb.tile([BC, 1536], F32)
                for gi, wins in enumerate(GROUPS):
                    nwin = len(wins)
                    nw = nwin * 128
                    phiT = {k: ptp.tile([128, 512], F32R, tag=f"phiT{k}", name=f"phiT{k}_{gi}")
                            for k in range(KT)}
                    for wloc, win in enumerate(wins):
                        if win in POOL_WINDOWS:
                            st, scr, eng = state_pool, scr_p, "pool"
                        else:
                            st, scr, eng = state_dve, scr_d, "dve"
                        scan_window(nc, eng, st, delta[:], x0_sb[:], win, scr, tcg[win])
                        for k in range(KT):
                            ps = tpp.tile([128, 128], F32, tag="tp")
                            nc.tensor.transpose(ps[:], st[:, k * 128:(k + 1) * 128], ident[:])
                            nc.scalar.copy(phiT[k][:, wloc * 128:(wloc + 1) * 128], ps[:])
                    for net in range(2):
                        for gc in range(16):
                            wt = wtp.tile([128, KT * 128], F32R, tag="wt")
                            nc.sync.dma_start(wt[:], wf_d[net, gc])
                            psx = xpp.tile([128, 512], F32, tag="psx")
                            for k in range(KT):
                                nc.tensor.matmul(psx[:, :nw], wt[:, k * 128:(k + 1) * 128],
                                                 phiT[k][:, :nw],
                                                 start=(k == 0), stop=(k == KT - 1))
                            xs = xsp.tile([128, 512], F32, tag="xs")
                            nc.scalar.copy(xs[:, :nw], psx[:, :nw])
                            nc.sync.dma_start(
                                xw_d[net, wins[0]:wins[0] + nwin, :, gc * 128:(gc + 1) * 128]
                                    .rearrange("w p c -> p w c"),
                                xs[:, :nw].rearrange("p (w c) -> p w c", w=nwin, c=128))

            # ---------- phase D: recurrence
            whh_sb = lp.tile([128, 2 * 4 * G4], BF16)
            for net in range(2):
                for j in range(4):
                    nc.sync.dma_start(whh_sb[:, (net * 4 + j) * G4:(net * 4 + j + 1) * G4],
                                      whh_d[net, j])
            cT = lp.tile([128, 2 * 512], F32)
            hTb = lp.tile([128, 2 * 512], BF16)
            nc.gpsimd.memset(cT[:], 0.0)
            with tc.tile_pool(name="phd", bufs=3) as pd, \
                 tc.tile_pool(name="hst", bufs=4) as hst, \
                 tc.tile_pool(name="rps", bufs=2, space="PSUM") as rpp:
                for w in range(W):
                    for net in range(2):
                        xwt = pd.tile([128, G4], F32, tag="xw")
                        nc.sync.dma_start(xwt[:], xw_d[net, w])
                        if w > 0:
                            psg = rpp.tile([128, G4], F32, tag="psg")
                            for gc in range(16):
                                for j in range(4):
                                    base = (net * 4 + j) * G4
                                    nc.tensor.matmul(
                                        psg[:, gc * 128:(gc + 1) * 128],
                                        whh_sb[:, base + gc * 128:base + (gc + 1) * 128],
                                        hTb[:, net * 512 + j * 128:net * 512 + (j + 1) * 128],
                                        start=(j == 0), stop=(j == 3))
                            gats = pd.tile([128, G4], F32, tag="gats")
                            nc.vector.tensor_tensor(gats[:], psg[:], xwt[:], AL.add)
                        else:
                            gats = xwt
                        acts = pd.tile([128, G4], F32, tag="acts")
                        nc.scalar.activation(acts[:, 0:1024], gats[:, 0:1024], AF.Sigmoid)
                        nc.scalar.activation(acts[:, 1024:1536], gats[:, 1024:1536], AF.Tanh)
                        nc.scalar.activation(acts[:, 1536:2048], gats[:, 1536:2048], AF.Sigmoid)
                        co = net * 512
                        cs = cT[:, co:co + 512]
                        nc.vector.tensor_tensor(cs, acts[:, 512:1024], cs, AL.mult)
                        tmpc = hst.tile([128, 512], F32, tag="tmpc")
                        nc.vector.tensor_tensor(tmpc[:], acts[:, 0:512], acts[:, 1024:1536], AL.mult)
                        nc.vector.tensor_tensor(cs, cs, tmpc[:], AL.add)
                        th = hst.tile([128, 512], F32, tag="th")
                        nc.scalar.activation(th[:], cs, AF.Tanh)
                        hT = hst.tile([128, 512], F32R, tag="hT")
                        nc.vector.tensor_tensor(hT[:], acts[:, 1536:2048], th[:], AL.mult)
                        nc.vector.tensor_copy(hTb[:, co:co + 512], hT[:].bitcast(F32))
                        for j in range(4):
                            nc.sync.dma_start(hT_d[net, j][:, w * 128:(w + 1) * 128],
                                              hT[:, j * 128:(j + 1) * 128])

            # ---------- phase E: FFN heads
            nch = ((0, 512), (512, 1024), (1024, NWS))
            b01sb = lp.tile([128, 16], F32)
            nc.sync.dma_start(b01sb[:], b01_d[:])
            b2fsb = lp.tile([1, 1], F32)
            nc.sync.dma_start(b2fsb[:], b2f_d[:])
            b2gsb = lp.tile([8, 1], F32)
            nc.sync.dma_start(b2gsb[:], b2g_d[:])
            with tc.tile_pool(name="phe", bufs=1) as pe, \
                 tc.tile_pool(name="fps", bufs=2, space="PSUM") as fpp:
                for net in range(2):
                    hsb = pe.tile([128, 4 * NWS], F32R, tag="hsb")
                    for j in range(4):
                        nc.sync.dma_start(hsb[:, j * NWS:(j + 1) * NWS], hT_d[net, j])
                    w0sb = pe.tile([128, 4 * H], F32R, tag="w0sb")
                    w1sb = pe.tile([128, 4 * H], F32R, tag="w1sb")
                    for j in range(4):
                        nc.sync.dma_start(w0sb[:, j * H:(j + 1) * H], w0_d[net, j])
                        nc.sync.dma_start(w1sb[:, j * H:(j + 1) * H], w1_d[net, j])
                    o1 = pe.tile([128, 4 * NWS], F32R, tag="o1")
                    for oc in range(4):
                        for (n0, n1) in nch:
                            ps1 = fpp.tile([128, 512], F32, tag="ps1")
                            for j in range(4):
                                nc.tensor.matmul(ps1[:, :n1 - n0],
                                                 w0sb[:, j * H + oc * 128:j * H + (oc + 1) * 128],
                                                 hsb[:, j * NWS + n0:j * NWS + n1],
                                                 start=(j == 0), stop=(j == 3))
                            nc.scalar.activation(o1[:, oc * NWS + n0:oc * NWS + n1],
                                                 ps1[:, :n1 - n0], AF.Relu,
                                                 bias=b01sb[:, net * 4 + oc:net * 4 + oc + 1])
                    o2 = pe.tile([128, 4 * NWS], F32R, tag="o2")
                    for oc in range(4):
                        for (n0, n1) in nch:
                            ps2 = fpp.tile([128, 512], F32, tag="ps2")
                            for j in range(4):
                                nc.tensor.matmul(ps2[:, :n1 - n0],
                                                 w1sb[:, j * H + oc * 128:j * H + (oc + 1) * 128],
                                                 o1[:, j * NWS + n0:j * NWS + n1],
                                                 start=(j == 0), stop=(j == 3))
                            nc.scalar.activation(o2[:, oc * NWS + n0:oc * NWS + n1],
                                                 ps2[:, :n1 - n0], AF.Relu,
                                                 bias=b01sb[:, 8 + net * 4 + oc:8 + net * 4 + oc + 1])
                    if net == 0:
                        w2sb = pe.tile([128, 4], F32R, tag="w2f")
                        for j in range(4):
                            nc.sync.dma_start(w2sb[:, j:j + 1], w2f_d[j])
                        for (n0, n1) in nch:
                            ps3 = fpp.tile([1, 512], F32, tag="ps3f")
                            for j in range(4):
                                nc.tensor.matmul(ps3[:, :n1 - n0], w2sb[:, j:j + 1],
                                                 o2[:, j * NWS + n0:j * NWS + n1],
                                                 start=(j == 0), stop=(j == 3))
                            nc.vector.tensor_scalar(yt[:, n0:n1], ps3[:, :n1 - n0],
                                                    b2fsb[:, 0:1], None, AL.add)
                    else:
                        w2sb = pe.tile([128, 4 * 8], F32R, tag="w2g")
                        for j in range(4):
                            nc.sync.dma_start(w2sb[:, j * 8:(j + 1) * 8], w2g_d[j])
                        for (n0, n1) in nch:
                            ps3 = fpp.tile([8, 512], F32, tag="ps3g")
                            for j in range(4):
                                nc.tensor.matmul(ps3[:, :n1 - n0], w2sb[:, j * 8:(j + 1) * 8],
                                                 o2[:, j * NWS + n0:j * NWS + n1],
                                                 start=(j == 0), stop=(j == 3))
                            nc.vector.tensor_scalar(zt[:, n0:n1], ps3[:, :n1 - n0],
                                                    b2gsb[:, 0:1], None, AL.add)
            nc.sync.dma_start(yt_d[:], yt[:])

            # ---------- phase F: tail
            with tc.tile_pool(name="phf", bufs=1) as pf, \
                 tc.tile_pool(name="tps2", bufs=2, space="PSUM") as tp2:
                sincT = pf.tile([8, NWS], F32)
                nc.gpsimd.memset(sincT[:, (W - 1) * 128:], 0.0)
                for w in range(W - 1):
                    pst = tp2.tile([8, 128], F32, tag="tps")
                    nc.tensor.transpose(pst[:], sinc[:, w * D:(w + 1) * D], ident[:])
                    nc.scalar.copy(sincT[:, w * 128:(w + 1) * 128], pst[:])
                zs = pf.tile([8, NWS], F32R)
                nc.vector.tensor_tensor(zs[:], zt[:], sincT[:], AL.mult)
                ones8f = pf.tile([8, 1], F32)
                nc.gpsimd.memset(ones8f[:], 1.0)
                ones8 = pf.tile([8, 1], F32R)
                nc.vector.tensor_copy(ones8[:], ones8f[:])
                pred = pf.tile([1, NWS], F32)
                for (n0, n1) in ((0, 512), (512, 1024), (1024, NWS)):
                    psz = tp2.tile([1, 512], F32, tag="psz")
                    nc.tensor.matmul(psz[:, :n1 - n0], ones8[:], zs[:, n0:n1],
                                     start=True, stop=True)
                    nc.vector.tensor_tensor(pred[:, n0:n1], yt[:, n0:n1],
                                            psz[:, :n1 - n0], AL.add)
                targ = pf.tile([1, NWS], F32)
                for w in range(W - 1):
                    nc.vector.tensor_scalar_mul(targ[:, w * 128:(w + 1) * 128],
                                                yt[:, (w + 1) * 128:(w + 2) * 128], float(disc[w]))
                pstp = tp2.tile([1, 128], F32, tag="tpp2")
                nc.tensor.transpose(pstp[:], pay[:], ident[:])
                nc.scalar.copy(targ[:, (W - 1) * 128:], pstp[:])
                diff = pf.tile([1, NWS], F32)
                nc.vector.tensor_tensor(diff[:], pred[:], targ[:], AL.subtract)
                nc.vector.tensor_tensor(diff[:], diff[:], diff[:], AL.mult)
                wsq = pf.tile([1, W], F32)
                nc.vector.tensor_reduce(wsq[:], diff[:].rearrange("p (w s) -> p w s", w=W),
                                        mybir.AxisListType.X, AL.add)
                nc.sync.dma_start(wsq_d[:], wsq[:])

    patch_nc(nc)
    return nc


_CACHE = {}


def _host_prep(inputs):
    import ml_dtypes
    ts = np.asarray(inputs["ts"], np.float32)
    h_t = ts[1:] - ts[:-1]
    sqrt_h = np.sqrt(h_t)
    tcg = ts[::LAG]
    disc = np.exp(-MU * (ts[np.arange(W - 1) + LAG] - tcg[:W - 1])).astype(np.float64)

    wf_host = np.zeros((2, 16, 128, KT * 128), np.float32)
    whh_host = np.zeros((2, 4, 128, G4), np.float32)
    w0_host = np.zeros((2, 4, 128, H), np.float32)
    w1_host = np.zeros((2, 4, 128, H), np.float32)
    b01_host = np.zeros((128, 16), np.float32)
    for n, pre in enumerate(("f", "g")):
        Wih = np.asarray(inputs[pre + "_Wih"], np.float32)
        Wf = fold_wih(Wih)
        Wf[:, NPHI - 2] = (np.asarray(inputs[pre + "_bih"], np.float32)
                           + np.asarray(inputs[pre + "_bhh"], np.float32))
        Wf[:, NPHI - 1] = Wih[:, 0]
        v = Wf.reshape(16, 128, KT, 128)                      # [gc][c][j][p]
        wf_host[n] = np.ascontiguousarray(v.transpose(0, 3, 2, 1)).reshape(16, 128, KT * 128)
        whh_host[n] = np.asarray(inputs[pre + "_Whh"], np.float32).T.reshape(4, 128, G4)
        w0_host[n] = np.asarray(inputs[pre + "_W0"], np.float32).T.reshape(4, 128, H)
        w1_host[n] = np.asarray(inputs[pre + "_W1"], np.float32).T.reshape(4, 128, H)
        b01_host[:, n * 4:(n + 1) * 4] = np.asarray(inputs[pre + "_b0"], np.float32).reshape(4, 128).T
        b01_host[:, 8 + n * 4:8 + (n + 1) * 4] = np.asarray(inputs[pre + "_b1"], np.float32).reshape(4, 128).T
    whh_host = whh_host.astype(ml_dtypes.bfloat16)
    w2f_host = np.ascontiguousarray(np.asarray(inputs["f_W2"], np.float32).T.reshape(4, 128, 1))
    w2g_host = np.ascontiguousarray(np.asarray(inputs["g_W2"], np.float32).T).reshape(4, 128, 8)
    b2f_host = np.asarray(inputs["f_b2"], np.float32).reshape(1, 1)
    b2g_host = np.asarray(inputs["g_b2"], np.float32).reshape(8, 1)

    crow = np.zeros((BC, 2 * NS * D), np.float32)
    crow[:, :NS * D] = np.repeat(sqrt_h, D)[None, :]
    crow[:, NS * D:] = np.repeat(1.0 + MU * h_t, D)[None, :]

    shared = dict(crows=crow, wf=wf_host, whh=whh_host,
                  w0=w0_host, w1=w1_host, w2f=w2f_host, w2g=w2g_host,
                  b01=b01_host, b2f=b2f_host, b2g=b2g_host)
    return disc, tcg, shared


def kernel(**inputs):
    x0 = np.ascontiguousarray(np.asarray(inputs["x0"], np.float32))
    noise = np.ascontiguousarray(np.asarray(inputs["noise"], np.float32))
    disc, tcg, shared = _host_prep(inputs)

    if "prog" not in _CACHE:
        _CACHE["prog"] = build_program(disc, tcg)
    nc = _CACHE["prog"]

    in_maps = []
    for c in range(NC):
        sl = slice(c * BC, (c + 1) * BC)
        in_maps.append(dict(shared, noise=noise[sl].reshape(BC, NS * D), x0=x0[sl]))

    res = run_bass_kernel_spmd(nc, in_maps, core_ids=list(range(NC)))

    Y = np.zeros((B, W, 1), np.float32)
    pay = np.zeros((B, 1), np.float32)
    wsq = np.zeros(W, np.float64)
    for c in range(NC):
        r = res.results[c]
        Y[c * BC:(c + 1) * BC] = r["yt"].reshape(W, BC).T[:, :, None]
        pay[c * BC:(c + 1) * BC] = r["pay"]
        wsq += r["wsq"][0].astype(np.float64)
    loss = np.float32((wsq / B).sum())
    return loss, Y, pay
